# revision 40
# baseline (speedup 1.0000x reference)
"""Trainium2 Bass kernel for the additive-attention problem.

reference math:
    rec[b,h]    = sum_r rnn_state[b,r] * W_rec[h,r]
    scores[t,b] = sum_h tanh(enc[t,b,h] + rec[b,h]) * w_score[h] + b_score + mask[t,b]
    out         = softmax(scores, axis=t)          # (T, B) float32

Sharding: data-parallel over B across 8 cores (BL=4 batch columns per core).
Softmax is over T (core-local), so no collectives.

Design (h-major layout, enc pre-staged on host as bf16):
  - host stages enc as [granule=256 t-rows][p=h%128][hc][b][t] bf16; tiles
    are 1-2 granules (TSCHED) -> 1MB DMAs, small first/last tiles for fast
    pipeline fill/drain, big tanh instrs mid-kernel (ScalarE ACTIVATE pays
    ~354ns fixed per instr + 0.826ns/elem and is the kernel bottleneck at
    ~57us busy; it runs gap-free in steady state).
  - concurrent DMAs fair-share SDMA bandwidth, so the fill phase is paced:
    granules 1-2 gate on the wrec load, granules 3-8 ladder on granule k-2
    (two transfers in flight) so arrivals stagger to match tanh cadence.
  - rec computed on device (16 small bf16 matmuls); in h-major layout rec is
    a per-partition scalar per (hc,b) slice -> VectorE tensor_scalar_add.
  - activation table pre-warmed with a dummy tanh so the ~1.5us
    ACT_TABLE_LOAD overlaps the first DMAs.
  - mask is pre-added INTO the PSUM scores tile by an identity-matmul before
    the main loop (start=True); all score matmuls then accumulate onto it
    (start=False, per-element has_written semantics).
  - score reduction over h on TensorE: per 128 consecutive t (fixed b),
    lhsT = tanh-slice (p=h, m=t) stationary, rhs = w chunk (128,1) bf16,
    accumulated over 4 h-chunks into one column of the persistent PSUM
    scores tile (p=t%128, f=(t//128, b)).
  - epilogue: ScalarE exp (PSUM src) -> one matmul with rhs=[I|ones] doing
    transpose AND row sums together -> m4 block-mask matmul broadcasts
    per-b totals -> reciprocal -> scale -> output DMA as (BL,T) 512B runs.
b_score cancels in softmax and is ignored.  No max-subtraction needed:
|scores| <= ||w_score||_1 <~ 25, safely inside f32 exp range.
"""

import numpy as np

T, B, H, R = 4096, 32, 512, 512
NCORES = 8
BL = B // NCORES          # 4 local batch columns
GT = 256                  # granule t rows (host staging unit)
NG = T // GT              # 16 granules
HC = H // 128             # 4 h-chunks
RC = R // 128             # 4 r-chunks
# tiles in granules: small first tiles for pipeline fill, 512-row steady
TSCHED = [1, 1, 1, 1, 2, 2, 2, 2, 2, 1, 1]
assert sum(TSCHED) == NG

_GRAPH = None


def _build_graph():
    import concourse.tile as tile
    from concourse import bacc, mybir
    from concourse.masks import make_identity

    f32 = mybir.dt.float32
    bf16 = mybir.dt.bfloat16
    nc = bacc.Bacc()

    encA = nc.declare_dram_parameter(
        "encA", [4, 128, HC, BL, 256], bf16, isOutput=False
    )
    encB = nc.declare_dram_parameter(
        "encB", [5, 128, HC, BL, 512], bf16, isOutput=False
    )
    encC = nc.declare_dram_parameter(
        "encC", [1, 128, HC, BL, 256], bf16, isOutput=False
    )
    encD = nc.declare_dram_parameter(
        "encD", [2, 128, HC, BL, 128], bf16, isOutput=False
    )
    # W_rec chunks and rnn columns packed into one tensor -> ONE DMA on the
    # critical rec-chain path instead of two issue slots
    wrbT = nc.declare_dram_parameter(
        "wrbT", [RC, 128, H + BL], bf16, isOutput=False
    )
    # packed small inputs: cols 0:128 m4, 128:256 mask (p=t%128, f=(a,b)),
    # 256:260 w_score chunks as f32 -> ONE small DMA on the sync ring
    packd = nc.declare_dram_parameter("packd", [128, 260], f32, isOutput=False)
    out = nc.declare_dram_parameter("out", [BL, T], bf16, isOutput=True)

    with tile.TileContext(nc) as tc:
        with (
            tc.tile_pool(name="singles", bufs=1) as singles,
            tc.tile_pool(name="xpool", bufs=4) as xpool,
            tc.tile_pool(name="ypool", bufs=2) as ypool,
            tc.tile_pool(name="spsum", bufs=1, space="PSUM") as spsum,
            tc.tile_pool(name="epsum", bufs=2, space="PSUM") as epsum,
        ):
            encAv = encA.rearrange("g p c b t -> g p c b t")
            encBv = encB.rearrange("g p c b t -> g p c b t")
            encCv = encC.rearrange("g p c b t -> g p c b t")
            encDv = encD.rearrange("g p c b t -> g p c b t")
            def enc_src(k):
                if k < 4:
                    return encAv[k]
                if k < 9:
                    return encBv[k - 4]
                if k == 9:
                    return encCv[0]
                return encDv[k - 10]

            # identity+ones for the combined transpose/row-sum matmul, and
            # an early dummy tanh to pull ACT_TABLE_LOAD off the critical path
            idf = singles.tile([128, 128], f32)
            make_identity(nc, idf[:])
            idext = singles.tile([128, 129], bf16)
            nc.vector.tensor_copy(out=idext[:, 0:128], in_=idf[:])
            nc.gpsimd.memset(idext[:, 128:129], 1.0)
            warm = singles.tile([128, 1], f32)
            nc.scalar.activation(
                out=warm[:], in_=idf[:, 0:1],
                func=mybir.ActivationFunctionType.Tanh,
            )

            # granule 0 goes FIRST on the sync ring (earliest position in the
            # DMA completion-sem lane, so the V adds' wait fires at its true
            # completion); rec-chain inputs right behind it.  Granules 1+ are
            # gated on the wrec load (add_dep below) so the prefetch burst
            # cannot starve the rec chain.
            X0 = xpool.tile([128, HC, BL, 256], bf16)
            g0_dma = nc.sync.dma_start(out=X0[:], in_=enc_src(0))

            wrb_sb = singles.tile([128, RC, H + BL], bf16)
            wrec_dma = nc.sync.dma_start(
                out=wrb_sb[:], in_=wrbT.rearrange("r p h -> p r h")
            )

            pack = singles.tile([128, 260], f32)
            nc.sync.dma_start(out=pack[:], in_=packd[:])
            mask_flat = pack[:, 128:256]
            m4 = singles.tile([128, 128], bf16)
            nc.vector.tensor_copy(out=m4[:], in_=pack[:, 0:128])
            w_sb = singles.tile([128, HC], bf16)
            nc.vector.tensor_copy(out=w_sb[:], in_=pack[:, 256:260])

            # ---------- rec[h, b] = sum_r W_rec[h,r] rnn[b,r] ----------
            rec_ps = epsum.tile([128, HC, BL], f32, tag="epi")
            for hc in range(HC):
                for rc in range(RC):
                    nc.tensor.matmul(
                        rec_ps[:, hc, :],
                        lhsT=wrb_sb[:, rc, hc * 128 : (hc + 1) * 128],
                        rhs=wrb_sb[:, rc, H : H + BL],
                        start=(rc == 0),
                        stop=(rc == RC - 1),
                    )

            # single copy AFTER all rec matmuls: VectorE must not read the
            # rec PSUM bank while the PE is still writing other columns of it
            # (PE-write + DVE-read of one bank is a hardware race)
            rec_sb = singles.tile([128, HC, BL], f32)
            nc.vector.tensor_copy(out=rec_sb[:], in_=rec_ps[:])

            # persistent scores accumulator: (p=t%128, f=(t//128, b));
            # seeded with the additive mask (identity matmul, start=True) so
            # every score matmul just accumulates (start=False).
            scores_ps = spsum.tile([128, NG * (GT // 128) * BL], f32)
            nc.tensor.matmul(
                scores_ps[:],
                lhsT=idf[:],
                rhs=mask_flat,
                start=True,
                stop=False,
                skip_group_check=True,
            )

            # ---------- main loop over t tiles ----------
            from concourse.tile_rust import add_dep_helper

            # fill-phase DMA pacing: tiles 1-2 gate on the wrec load,
            # tiles 3-8 ladder on tile k-2 (two transfers in flight) so
            # arrivals stagger instead of fair-sharing into a late bunch
            ROWS = [256, 256, 256, 256, 512, 512, 512, 512, 512, 256, 128, 128]
            t0_rows = 0
            tdma = {}
            for k, rows in enumerate(ROWS):
                if k == 0:
                    X = X0
                else:
                    X = xpool.tile([128, HC, BL, rows], bf16)
                    d = nc.sync.dma_start(out=X[:], in_=enc_src(k))
                    tdma[k] = d
                    if k == 1:
                        # g0 completes ~1us before wrec; gating tile1 on it
                        # staggers the release and lands tile1 sooner
                        add_dep_helper(
                            d.ins, g0_dma.ins, sync=True,
                            reason="keep startup burst off the rec chain",
                        )
                    elif k == 2:
                        add_dep_helper(
                            d.ins, wrec_dma.ins, sync=True,
                            reason="keep startup burst off the rec chain",
                        )
                    elif 3 <= k <= 8:
                        add_dep_helper(
                            d.ins, tdma[k - 2].ins, sync=True,
                            reason="stagger fill-phase tile arrivals",
                        )
                for hc in range(HC):
                    for b in range(BL):
                        nc.vector.tensor_scalar_add(
                            out=X[:, hc, b, :],
                            in0=X[:, hc, b, :],
                            scalar1=rec_sb[:, hc, b : b + 1],
                        )
                Y = ypool.tile([128, HC, BL, rows], bf16)
                nc.scalar.activation(
                    out=Y[:],
                    in_=X[:],
                    func=mybir.ActivationFunctionType.Tanh,
                )
                for ts in range(rows // 128):
                    a = t0_rows // 128 + ts
                    for b in range(BL):
                        c = a * BL + b
                        for hc in range(HC):
                            nc.tensor.matmul(
                                scores_ps[:, c : c + 1],
                                lhsT=Y[:, hc, b, ts * 128 : (ts + 1) * 128],
                                rhs=w_sb[:, hc : hc + 1],
                                start=False,
                                stop=(hc == HC - 1),
                                skip_group_check=True,
                            )
                t0_rows += rows

            # ---------- exp, transpose+row-sums, normalize, output ----------
            E = singles.tile([128, 128], bf16)
            nc.scalar.activation(
                out=E[:], in_=scores_ps[:], func=mybir.ActivationFunctionType.Exp
            )
            # one matmul: cols 0..127 = E^T (p=(a,b), f=t%128), col 128 = row sums
            attx = epsum.tile([128, 129], f32, tag="epi")
            nc.tensor.matmul(
                attx[:], lhsT=E[:], rhs=idext[:], start=True, stop=True
            )
            rs_sb = singles.tile([128, 1], bf16)
            nc.vector.tensor_copy(out=rs_sb[:], in_=attx[:, 128:129])
            denom = epsum.tile([128, 1], f32, tag="epi")
            nc.tensor.matmul(
                denom[:], lhsT=m4[:], rhs=rs_sb[:], start=True, stop=True
            )
            recip = singles.tile([128, 1], f32)
            nc.vector.reciprocal(out=recip[:], in_=denom[:])
            # bf16 output: faster final scale and a 32KB (not 64KB) out DMA;
            # host casts back to f32 (att weights are in [0,1], 0.4% rounding)
            att_out = singles.tile([128, 128], bf16)
            nc.vector.tensor_scalar_mul(
                out=att_out[:], in0=attx[:, 0:128], scalar1=recip[:]
            )
            # partition p = (a=t//128, b) holds 128 contiguous t values for col b
            nc.sync.dma_start(
                out=out.rearrange("b (a tp) -> a b tp", tp=128),
                in_=att_out[:],
            )

    nc.compile()
    return nc


def _get_graph():
    global _GRAPH
    if _GRAPH is None:
        _GRAPH = _build_graph()
    return _GRAPH


def make_in_maps(enc, mask, rnn_state, W_rec, w_score):
    import ml_dtypes

    bf = ml_dtypes.bfloat16
    enc_bf = np.asarray(enc, dtype=np.float32).astype(bf)
    # [g, t, core, b, hc, p] view -> per-core [g, p, hc, b, t] (256-row units)
    e6 = enc_bf.reshape(NG, GT, NCORES, BL, HC, 128).transpose(2, 0, 5, 4, 3, 1)
    wrecT = W_rec.T.astype(np.float32).astype(bf).reshape(RC, 128, H)
    m4 = (np.arange(128)[:, None] % BL == np.arange(128)[None, :] % BL).astype(
        np.float32
    )
    mask = np.asarray(mask, dtype=np.float32)
    rnn = np.asarray(rnn_state, dtype=np.float32)
    wf = w_score.astype(np.float32).reshape(HC, 128).T  # (128, HC)
    in_maps = []
    for c in range(NCORES):
        sl = slice(c * BL, (c + 1) * BL)
        # mask in (p=t%128, f=(a=t//128, b)) layout, flattened to 128 cols
        mcols = np.ascontiguousarray(
            mask[:, sl].reshape(NG * (GT // 128), 128, BL).transpose(1, 0, 2)
        ).reshape(128, 128)
        packd = np.concatenate([m4, mcols, wf], axis=1).astype(np.float32)
        in_maps.append(
            {
                "encA": np.ascontiguousarray(e6[c, 0:4]),
                "encB": np.ascontiguousarray(
                    e6[c, 4:14].reshape(5, 2, 128, HC, BL, 256)
                    .transpose(0, 2, 3, 4, 1, 5)
                    .reshape(5, 128, HC, BL, 512)
                ),
                "encC": np.ascontiguousarray(e6[c, 14:15]),
                "encD": np.ascontiguousarray(
                    e6[c, 15].reshape(128, HC, BL, 2, 128)
                    .transpose(3, 0, 1, 2, 4)
                ),
                "wrbT": np.ascontiguousarray(
                    np.concatenate(
                        [wrecT, rnn[sl].T.astype(bf).reshape(RC, 128, BL)],
                        axis=2,
                    )
                ),
                "packd": np.ascontiguousarray(packd),
            }
        )
    return in_maps


def kernel(
    encoded_contribution,
    mask,
    rnn_state,
    prev_att_weights,
    W_rec,
    w_score,
    b_score,
):
    from concourse.bass_utils import run_bass_kernel_spmd

    nc = _get_graph()
    in_maps = make_in_maps(
        np.asarray(encoded_contribution),
        np.asarray(mask),
        np.asarray(rnn_state),
        np.asarray(W_rec),
        np.asarray(w_score),
    )
    res = run_bass_kernel_spmd(nc, in_maps, list(range(NCORES)))
    outs = [np.asarray(res.results[c]["out"]) for c in range(NCORES)]
    return np.concatenate([o.T for o in outs], axis=1).astype(np.float32)
